# revision 28
# baseline (speedup 1.0000x reference)
"""AttentionBlock (GroupNorm + 4-head self-attention + proj + residual) on 8 trn2 cores.

Sharding: core i handles (batch b = i//4, query-chunk j = i%4, TQ=1024).
Each core gets batch b's x rotated so its query chunk sits at columns 0:1024.

Design:
- x loaded in chunks over sync+scalar+gpsimd DMA queues; bn_stats streamed per
  chunk; GroupNorm folded into qkv weights (alpha) and a bias column (beta),
  cross-partition reshapes done via N=1 matmuls (no DRAM round-trips).
- Scores bf16, row-tiled pairs (two heads concurrently on the PE array).
- exp split scalar (AF.Exp -> fp8e4, scaled 2^-3 to stay under fp8 max-finite
  240) / vector (uint8 bitcast trick); per-block lane ratios tuned.
- PV in fp8 DoubleRow: contraction 256 (two 128-key tiles) per matmul at
  2 cols/cycle, ones column (65th) accumulates the softmax denominator.
- Softmax reciprocal broadcast via gpsimd partition_broadcast (SBUF only).
- Single PSUM pool for the whole kernel: tag "s" (3x [128,1024] rotating, also
  used by qkv/v/bias/proj psums) + tag "pv" (2x [65,512]), so phases interleave
  without pool-boundary serialization. v and k(pair1) matmuls are emitted
  inside block 1's stream; each block's normalize/proj is emitted a few steps
  into the next block to avoid head-of-line blocking on the in-order queues.
"""
import sys

if "/opt/trn_rl_repo" not in sys.path:
    sys.path.insert(0, "/opt/trn_rl_repo")

import numpy as np
import ml_dtypes

import concourse.bass as bass
import concourse.bacc as bacc
import concourse.tile as tile
from concourse import mybir
from concourse.bass_utils import run_bass_kernel_spmd

B, C, T = 2, 256, 4096
NH, CH = 4, 64
TQ = 1024
P = 128
EPS = 1e-5
SCALE = float(1.0 / np.sqrt(np.sqrt(np.float32(CH))))

F32 = mybir.dt.float32
F32R = mybir.dt.float32r
BF16 = mybir.dt.bfloat16
FP8 = mybir.dt.float8e4
U8 = mybir.dt.uint8
AF = mybir.ActivationFunctionType
ALU = mybir.AluOpType
DR = mybir.MatmulPerfMode.DoubleRow

# exp->fp8e4 bitcast: byte = round(8/ln2 * s + (7<<3 - 0.35) - 24); es scaled
# 2^-3 so values stay below fp8e4 max-finite 240 (0x78 is +inf on the PE).
EXP_A8 = float(8.0 / np.log(2.0))
EXP_B8 = 55.65 - 24.0

def _vec_set(n):
    return frozenset(t for t in range(32) if (t * n) // 32 != ((t + 1) * n) // 32)

TRACE = False
LAST_RESULTS = None
_CACHE = {}


def _build_program():
    nc = bacc.Bacc("TRN2", target_bir_lowering=False, debug=False, num_devices=8)
    d = {}
    d["x"] = nc.dram_tensor("x", [C, T], F32R, kind="ExternalInput")
    d["wt_qkv"] = nc.dram_tensor("wt_qkv", [C, 3 * C], F32R, kind="ExternalInput")
    d["wt_proj"] = nc.dram_tensor("wt_proj", [C, C], BF16, kind="ExternalInput")
    d["gn2s"] = nc.dram_tensor("gn2s", [P, 2], F32, kind="ExternalInput")
    d["gn2b"] = nc.dram_tensor("gn2b", [P, 2], F32, kind="ExternalInput")
    d["bq6"] = nc.dram_tensor("bq6", [P, 6], F32, kind="ExternalInput")
    d["bpj2"] = nc.dram_tensor("bpj2", [P, 2], F32, kind="ExternalInput")
    d["gmat"] = nc.dram_tensor("gmat", [P, P], F32, kind="ExternalInput")
    d["out"] = nc.dram_tensor("out", [C, TQ], F32, kind="ExternalOutput")

    with tile.TileContext(nc) as tc:
        _body(tc, nc, d)
    nc.compile()
    return nc


def _body(tc, nc, d):
    from contextlib import ExitStack

    ctx = ExitStack()
    with ctx:
        const1 = ctx.enter_context(tc.tile_pool(name="const", bufs=1))
        xpool = ctx.enter_context(tc.tile_pool(name="xp", bufs=1))
        wpool = ctx.enter_context(tc.tile_pool(name="wp", bufs=1))
        kqv = ctx.enter_context(tc.tile_pool(name="kqv", bufs=1))
        small = ctx.enter_context(tc.tile_pool(name="small", bufs=4))
        epool = ctx.enter_context(tc.tile_pool(name="expp", bufs=6))
        opool = ctx.enter_context(tc.tile_pool(name="op", bufs=2))
        psum = ctx.enter_context(tc.tile_pool(name="ps", bufs=3, space="PSUM"))

        def sps(name, cols=1024):
            return psum.tile([P, cols], F32, tag="s", bufs=3, name=name)

        # ---- loads: x chunks on sync+scalar, weights+small on gpsimd ----
        xt = [xpool.tile([P, T], F32R, tag=f"x{t}", name=f"x{t}") for t in range(2)]
        for chk in range(3):
            lo = chk * 1024
            nc.sync.dma_start(xt[0][:, lo:lo + 1024], d["x"][0:P, lo:lo + 1024])
            nc.scalar.dma_start(xt[1][:, lo:lo + 1024], d["x"][P:2 * P, lo:lo + 1024])
        nc.gpsimd.dma_start(xt[0][:, 3072:4096], d["x"][0:P, 3072:4096])
        gn2s = const1.tile([P, 2], F32, tag="gn2s")
        nc.gpsimd.dma_start(gn2s[:], d["gn2s"][:, :])
        gn2b = const1.tile([P, 2], F32, tag="gn2b")
        nc.gpsimd.dma_start(gn2b[:], d["gn2b"][:, :])
        bq6 = const1.tile([P, 6], F32, tag="bq6")
        nc.gpsimd.dma_start(bq6[:], d["bq6"][:, :])
        bpj2 = const1.tile([P, 2], F32, tag="bpj2")
        nc.gpsimd.dma_start(bpj2[:], d["bpj2"][:, :])
        gmat = const1.tile([P, P], F32, tag="gmat")
        nc.gpsimd.dma_start(gmat[:], d["gmat"][:, :])
        wt = []
        for t in range(2):
            wi = wpool.tile([P, 3 * C], F32R, tag=f"wt{t}", name=f"wt{t}")
            nc.gpsimd.dma_start(wi[:], d["wt_qkv"][t * P:(t + 1) * P, :])
            wt.append(wi)
        wtp = []
        for t in range(2):
            wi = wpool.tile([P, C], BF16, tag=f"wtp{t}", name=f"wtp{t}")
            nc.gpsimd.dma_start(wi[:], d["wt_proj"][t * P:(t + 1) * P, :])
            wtp.append(wi)
        # tail x chunks split small so stats can finish sooner
        nc.sync.dma_start(xt[1][:, 3072:3584], d["x"][P:2 * P, 3072:3584])
        nc.scalar.dma_start(xt[1][:, 3584:4096], d["x"][P:2 * P, 3584:4096])

        # HAM warmup: tiny matmuls spaced by x-chunk arrivals keep the PE
        # clock-gate open through the load phase.
        warm = psum.tile([P, 8], F32, tag="s", bufs=3, name="warm")
        for chk in range(3):
            for t in range(2):
                nc.tensor.matmul(warm[:, 0:4], lhsT=gmat[:],
                                 rhs=xt[t][:, chk * 1024:chk * 1024 + 4].bitcast(F32),
                                 start=True, stop=True)
        nc.tensor.matmul(warm[:, 0:4], lhsT=gmat[:],
                         rhs=xt[0][:, 3072:3076].bitcast(F32), start=True, stop=True)
        nc.tensor.matmul(warm[:, 0:4], lhsT=gmat[:],
                         rhs=xt[1][:, 3584:3588].bitcast(F32), start=True, stop=True)
        # warm the es pool: memset each rotating buffer once during the load
        # window so a too-early PV read can never see uninitialized fp8 bytes
        # (0x78 = +inf on the PE would poison the accumulators).
        for i in range(6):
            es_w = epool.tile([P, 2, 1024], FP8, tag="exp", name=f"esw{i}")
            nc.vector.memset(es_w[:], 0.0)
        eps_t = const1.tile([P, 1], F32, tag="eps")
        nc.vector.memset(eps_t[:], EPS)
        expb = const1.tile([P, 1], F32, tag="expb")
        nc.vector.memset(expb[:], float(-3.0 * np.log(2.0)))

        # ---- streamed group stats (chunk-arrival order) ----
        st = [small.tile([P, 6, 6], F32, tag=f"bnst{t}", name=f"bnst{t}")
              for t in range(2)]
        for i in range(6):
            for t in range(2):
                xv = xt[t].rearrange("p (n f) -> p n f", f=512)
                nc.vector.bn_stats(st[t][:, i, :], xv[:, i, :])
        stats4 = small.tile([P, 4], F32, tag="stats4")
        for t in range(2):
            mv = small.tile([P, 2], F32, tag="mv")
            nc.vector.bn_aggr(mv[:], st[t][:])
            nc.vector.tensor_copy(stats4[:, 2 * t:2 * t + 1], mv[:, 0:1])
            msq = small.tile([P, 1], F32, tag="msq")
            nc.scalar.square(msq[:], mv[:, 0:1])
            nc.vector.tensor_add(stats4[:, 2 * t + 1:2 * t + 2], mv[:, 1:2], msq[:])

        # group-reduce via gmat; alpha/beta as [P, 2] (col = tile t)
        alpha = const1.tile([P, 2], F32, tag="alpha")
        beta = const1.tile([P, 2], F32, tag="beta")
        gsum = sps("gsum", 8)
        nc.tensor.matmul(gsum[:, 0:4], lhsT=gmat[:], rhs=stats4[:],
                         start=True, stop=True)
        ge = small.tile([P, 4], F32, tag="ge")
        nc.scalar.mul(ge[:], gsum[:, 0:4], 0.125)       # [m0, e0, m1, e1]
        mean = ge[:].rearrange("p (a b) -> p a b", b=2)[:, :, 0]
        e8 = ge[:].rearrange("p (a b) -> p a b", b=2)[:, :, 1]
        msq2 = small.tile([P, 2], F32, tag="msq2")
        nc.vector.tensor_mul(msq2[:], mean, mean)
        var = small.tile([P, 2], F32, tag="var")
        nc.vector.tensor_sub(var[:], e8, msq2[:])
        std = small.tile([P, 2], F32, tag="std")
        nc.scalar.activation(std[:], var[:], AF.Sqrt, bias=eps_t[:])
        rstd = small.tile([P, 2], F32, tag="rstd")
        nc.vector.reciprocal(rstd[:], std[:])
        nc.vector.tensor_mul(alpha[:], rstd[:], gn2s[:])
        tmp = small.tile([P, 2], F32, tag="tmpb")
        nc.vector.tensor_mul(tmp[:], mean, alpha[:])
        nc.vector.tensor_sub(beta[:], gn2b[:], tmp[:])

        # ---- fold alpha into weights ----
        wta = []
        for t in range(2):
            wi = wpool.tile([P, 3 * C], F32R, tag=f"wta{t}", name=f"wta{t}")
            if t == 0:
                nc.scalar.activation(wi[:], wt[t][:], AF.Copy, scale=alpha[:, 0:1])
            else:
                nc.vector.tensor_scalar_mul(wi[:], wt[t][:], alpha[:, 1:2])
            wta.append(wi)

        k_sb = [kqv.tile([P, T], BF16, tag=f"k{t}", name=f"k{t}") for t in range(2)]
        q_sb = [kqv.tile([P, TQ], BF16, tag=f"q{t}", name=f"q{t}") for t in range(2)]
        vT8 = [kqv.tile([P, 2, NH, 68], FP8, tag=f"v{r}", name=f"v{r}")
               for r in range(16)]
        for r in range(16):
            nc.vector.memset(vT8[r][:, :, :, 64:65], 1.0)
        a_sb = [kqv.tile([P, TQ], BF16, tag=f"a{t}", name=f"a{t}") for t in range(2)]


        bcol = const1.tile([P, 6], F32, tag="bcol")
        bcol_s = const1.tile([P, 4], F32, tag="bcols")
        vb2 = const1.tile([P, 2], BF16, tag="vb2")
        fb = const1.tile([P, 2], F32, tag="fb")

        def emit_k(p, tcn):
            ps = sps(f"kps{p}{tcn}")
            for half in range(2):
                col = slice(tcn * 1024 + half * 512, tcn * 1024 + half * 512 + 512)
                pcol = slice(half * 512, half * 512 + 512)
                for t in range(2):
                    nc.tensor.matmul(
                        ps[:, pcol],
                        lhsT=wta[t][:, 256 + p * P:256 + (p + 1) * P],
                        rhs=xt[t][:, col],
                        start=(t == 0), stop=(t == 1),
                    )
            dst = k_sb[p][:, tcn * 1024:(tcn + 1) * 1024]
            if tcn != 3:
                nc.scalar.activation(dst, ps[:], AF.Identity,
                                     bias=bcol_s[:, 2 + p:3 + p], scale=SCALE)
            else:
                nc.vector.tensor_scalar(out=dst, in0=ps[:], scalar1=SCALE,
                                        scalar2=bcol_s[:, 2 + p:3 + p],
                                        op0=ALU.mult, op1=ALU.add)

        def emit_bias():
            bias_ps = sps("bias_ps")
            for j in range(6):
                for t in range(2):
                    nc.tensor.matmul(
                        bias_ps[:, j:j + 1],
                        lhsT=wt[t][:, j * P:(j + 1) * P].bitcast(F32),
                        rhs=beta[:, t:t + 1],
                        start=(t == 0), stop=(t == 1),
                    )
            nc.vector.tensor_add(bcol[:], bias_ps[:, 0:6], bq6[:])
            nc.scalar.mul(bcol_s[:], bcol[:, 0:4], SCALE)
            nc.vector.tensor_copy(vb2[:], bcol[:, 4:6])

        def emit_q(ot):
            ps = sps(f"qps{ot}")
            for half in range(2):
                pcol = slice(half * 512, half * 512 + 512)
                for t in range(2):
                    nc.tensor.matmul(
                        ps[:, pcol],
                        lhsT=wta[t][:, ot * P:(ot + 1) * P],
                        rhs=xt[t][:, pcol],
                        start=(t == 0), stop=(t == 1),
                    )
            if ot == 0:
                nc.scalar.activation(q_sb[ot][:], ps[:], AF.Identity,
                                     bias=bcol_s[:, ot:ot + 1], scale=SCALE)
            else:
                nc.vector.tensor_scalar(out=q_sb[ot][:], in0=ps[:], scalar1=SCALE,
                                        scalar2=bcol_s[:, ot:ot + 1],
                                        op0=ALU.mult, op1=ALU.add)

        def emit_fb():
            fps = sps("fps", 8)
            for j in range(2):
                for t in range(2):
                    nc.tensor.matmul(
                        fps[:, j:j + 1],
                        lhsT=wtp[t][:, j * P:(j + 1) * P],
                        rhs=vb2[:, t:t + 1],
                        start=(t == 0), stop=(t == 1),
                    )
            nc.vector.tensor_add(fb[:], fps[:, 0:2], bpj2[:])

        def emit_v(r):
            ps = sps(f"vps{r}", 512)
            for half in range(2):
                tt = r * 2 + half
                for t in range(2):
                    nc.tensor.matmul(
                        ps[:, half * 256:(half + 1) * 256],
                        lhsT=xt[t][:, tt * P:(tt + 1) * P],
                        rhs=wta[t][:, 512:768],
                        start=(t == 0), stop=(t == 1),
                    )
            pv_view = ps[:].rearrange("p (k h c) -> p k h c", k=2, c=64)
            if r % 2 == 0:
                nc.scalar.activation(vT8[r][:, :, :, 0:64], pv_view, AF.Copy)
            else:
                nc.vector.tensor_copy(vT8[r][:, :, :, 0:64], pv_view)

        block_idx = [0]

        def do_block(p, c, vec_n=14, mm_hook=None, inject=None,
                     vec_last_scalar=False):
            kt = k_sb[p]
            qt = q_sb[p]
            vec_tt = set(_vec_set(vec_n))
            if vec_last_scalar:
                vec_tt.discard(31)
                vec_tt.add(30)
            cc = slice(c * 512, c * 512 + 512)
            psh = [psum.tile([65, 512], F32, tag="pv", bufs=2, name=f"ph{p}{c}{hh}")
                   for hh in range(2)]
            sc_q = []

            def emit_scores(tt):
                sc = sps("sc")
                nc.tensor.matmul(
                    sc[:, 0:512],
                    lhsT=kt[0:64, tt * P:(tt + 1) * P],
                    rhs=qt[0:64, cc], start=True, stop=True)
                nc.tensor.matmul(
                    sc[:, 512:1024],
                    lhsT=kt[64:128, tt * P:(tt + 1) * P],
                    rhs=qt[64:128, cc], start=True, stop=True)
                sc_q.append(sc)

            def emit_exp(tt, es):
                sc = sc_q.pop(0)
                dst = es[:, tt % 2, :]
                if tt in vec_tt:
                    nc.vector.tensor_scalar(
                        out=dst.bitcast(U8), in0=sc[:],
                        scalar1=EXP_A8, scalar2=EXP_B8,
                        op0=ALU.mult, op1=ALU.add)
                else:
                    nc.scalar.activation(dst, sc[:], AF.Exp, bias=expb[:])

            def emit_pv(r, es):
                for hh in range(2):
                    nc.tensor.matmul(
                        psh[hh][:],
                        lhsT=vT8[r][:, :, 2 * p + hh, 0:65],
                        rhs=es[:, :, hh * 512:hh * 512 + 512],
                        start=(r == 0), stop=(r == 15),
                        perf_mode=DR)

            pending = None
            for r in range(16):
                if mm_hook is not None:
                    mm_hook(r)
                es = epool.tile([P, 2, 1024], FP8, tag="exp", name="es")
                emit_scores(2 * r)
                emit_exp(2 * r, es)
                emit_scores(2 * r + 1)
                emit_exp(2 * r + 1, es)
                if pending is not None:
                    emit_pv(*pending)
                pending = (r, es)
                if r == 2 and inject is not None:
                    inject()
            emit_pv(*pending)
            block_idx[0] += 1

            def finalize(staggered=False):
                if staggered:
                    # per-head chains: lower latency for the final block's tail
                    for hh in range(2):
                        dn1 = small.tile([1, 512], F32, tag="dn1", name="dn1")
                        nc.vector.tensor_copy(dn1[:], psh[hh][64:65, :])
                        rc1 = small.tile([1, 512], F32, tag="rc1", name="rc1")
                        nc.vector.reciprocal_approx_fast(out=rc1[:], in_=dn1[:])
                        rb1 = small.tile([64, 512], F32, tag="rb1", name="rb1")
                        nc.gpsimd.partition_broadcast(rb1[:], rc1[0:1, :])
                        nc.vector.tensor_mul(a_sb[p][64 * hh:64 * hh + 64, cc],
                                             psh[hh][0:64, :], rb1[:])
                    return
                # merged wide ops: fewer vector instructions for mid blocks
                dn2 = small.tile([1, 1024], F32, tag="dn2", name="dn2")
                for hh in range(2):
                    nc.vector.tensor_copy(dn2[0:1, hh * 512:(hh + 1) * 512],
                                          psh[hh][64:65, :])
                rc2 = small.tile([1, 1024], F32, tag="rc2", name="rc2")
                nc.vector.reciprocal_approx_fast(out=rc2[:], in_=dn2[:])
                rb2 = small.tile([64, 1024], F32, tag="rb2", name="rb2")
                nc.gpsimd.partition_broadcast(rb2[:], rc2[0:1, :])
                for hh in range(2):
                    nc.vector.tensor_mul(
                        a_sb[p][64 * hh:64 * hh + 64, cc],
                        psh[hh][0:64, :],
                        rb2[:, hh * 512:(hh + 1) * 512])
            return finalize

        def do_proj(c, eng1, po=None, t_range=(0, 1)):
            cc = slice(c * 512, c * 512 + 512)
            if po is None:
                po = sps(f"po{c}")
            for ot in range(2):
                for t in t_range:
                    nc.tensor.matmul(
                        po[:, ot * 512:(ot + 1) * 512],
                        lhsT=wtp[t][:, ot * P:(ot + 1) * P],
                        rhs=a_sb[t][:, cc],
                        start=(t == 0), stop=(t == 1))
            for ot in range(2):
                osb = opool.tile([P, 512], F32, tag="osb")
                nc.vector.scalar_tensor_tensor(
                    out=osb[:], in0=po[:, ot * 512:(ot + 1) * 512],
                    scalar=fb[:, ot:ot + 1],
                    in1=xt[ot][:, cc], op0=ALU.add, op1=ALU.add)
                eng = nc.sync if ot == 0 else eng1
                eng.dma_start(d["out"][ot * P:(ot + 1) * P, cc], osb[:])

        # ---- qkv lead-in: k(p0)/q/v interleaved; k(p1) streams inside block 1
        emit_bias()
        emit_k(0, 0)
        emit_v(0)
        emit_k(0, 1)
        emit_v(1)
        emit_k(0, 2)
        emit_v(2)
        emit_k(0, 3)
        emit_v(3)
        emit_q(0)
        emit_v(4)
        emit_q(1)
        emit_v(5)
        emit_fb()
        for r in range(6, 16):
            emit_v(r)

        def b2_hook(r):
            # k(pair 1) streams inside block 2; first needed by block 3
            if 4 <= r <= 10 and r % 2 == 0:
                emit_k(1, (r - 4) // 2)

        po1 = [None]

        def b4_hook(r):
            if r == 14:
                po1[0] = sps("po1")
                for ot in range(2):
                    nc.tensor.matmul(
                        po1[0][:, ot * 512:(ot + 1) * 512],
                        lhsT=wtp[0][:, ot * P:(ot + 1) * P],
                        rhs=a_sb[0][:, 512:1024],
                        start=True, stop=False)

        fin00 = do_block(0, 0, vec_n=15)
        fin01 = do_block(0, 1, vec_n=14, mm_hook=b2_hook, inject=fin00)
        fin10 = do_block(1, 0, vec_n=13, inject=fin01)
        fin11 = do_block(1, 1, vec_n=14, vec_last_scalar=True,
                         inject=lambda: (fin10(), do_proj(0, nc.gpsimd)),
                         mm_hook=b4_hook)
        fin11(staggered=True)
        do_proj(1, nc.scalar, po=po1[0], t_range=(1,))


def _get_program():
    if "nc" not in _CACHE:
        _CACHE["nc"] = _build_program()
    return _CACHE["nc"]


def kernel(x, gn_scale, gn_bias, w_qkv, b_qkv, w_proj, b_proj):
    global LAST_RESULTS
    nc = _get_program()
    xf = np.ascontiguousarray(np.asarray(x, dtype=np.float32).reshape(B, C, T))
    # Reference (QKVAttentionLegacy) splits qkv per head: rows 192h..192h+191
    # are [q_h | k_h | v_h]. Permute to our [all q | all k | all v] layout.
    perm = np.concatenate([
        np.arange(NH * 3 * CH).reshape(NH, 3, CH)[:, p, :].reshape(-1)
        for p in range(3)])
    w_qkv = np.asarray(w_qkv, np.float32)[perm]
    b_qkv = np.asarray(b_qkv, np.float32)[perm]
    wt_qkv = np.ascontiguousarray(w_qkv.T)
    wt_proj = np.ascontiguousarray(np.asarray(w_proj, np.float32).T).astype(ml_dtypes.bfloat16)
    gn2s = np.ascontiguousarray(np.asarray(gn_scale, np.float32).reshape(2, P).T)
    gn2b = np.ascontiguousarray(np.asarray(gn_bias, np.float32).reshape(2, P).T)
    bq6 = np.ascontiguousarray(np.asarray(b_qkv, np.float32).reshape(6, P).T)
    bpj2 = np.ascontiguousarray(np.asarray(b_proj, np.float32).reshape(2, P).T)
    gmat = np.kron(np.eye(16, dtype=np.float32), np.ones((8, 8), np.float32))

    in_maps = []
    for core in range(8):
        b, j = core // 4, core % 4
        off = j * TQ
        if off:
            xrot = np.ascontiguousarray(
                np.concatenate([xf[b][:, off:], xf[b][:, :off]], axis=1))
        else:
            xrot = xf[b]
        in_maps.append({
            "x": xrot, "wt_qkv": wt_qkv, "wt_proj": wt_proj,
            "gn2s": gn2s, "gn2b": gn2b, "bq6": bq6, "bpj2": bpj2,
            "gmat": gmat,
        })

    LAST_RESULTS = run_bass_kernel_spmd(
        nc, in_maps, core_ids=list(range(8)), trace=TRACE)

    full = np.empty((B, C, T), np.float32)
    for core in range(8):
        b, j = core // 4, core % 4
        full[b][:, j * TQ:(j + 1) * TQ] = LAST_RESULTS.results[core]["out"]
    return full.reshape(B, C, 64, 64)
